# revision 1
# baseline (speedup 1.0000x reference)
"""GCN (3-layer graph conv + 3-layer MLP head) on 8 TRN2 NeuronCores.

Strategy (graph/1D-row parallel, per sharding hint):
  - Nodes are row-sharded across the 8 cores (6250 rows each).
  - Per layer: local GEMM support = g_prev @ W (node shard), AllGather the
    [50000,128] fp16 support table to every core, then each core aggregates
    its destination rows: for each 128-row destination block, gather the
    neighbor rows (dma_gather, int16 indices, table split at row 32768 so
    indices fit int16), build a one-hot scatter matrix S[e,dst]=val[e] on
    DVE from a host-precomputed (dst,val) stream, and accumulate
    aggT[feat,dst] += msgs[e,feat].T @ S[e,dst] on the tensor engine with
    f32 PSUM. Bias+ReLU+fp16-cast happens on ACT straight out of PSUM.
  - Everything stays feature-major (gT = [feat, node]) so no transposes are
    ever needed; the FC head runs the same way and the [2, n] logits are
    transposed back on the host.

Numerics: fp16 storage / f32 accumulation -> ~1.2e-3 norm rel err vs the
f32 reference (validated offline in numpy).
"""

import numpy as np

import concourse.bass as bass
import concourse.bacc as bacc
import concourse.mybir as mybir
import concourse.tile as tile
from concourse.bass_utils import run_bass_kernel_spmd

FP16 = mybir.dt.float16
F32 = mybir.dt.float32
I16 = mybir.dt.int16

N_NODES = 50000
N_CORES = 8
D = 128
SPLIT = 32768  # int16 gather-index limit: table rows >= SPLIT use a 2nd base


# ---------------------------------------------------------------------------
# Host-side schedule construction
# ---------------------------------------------------------------------------
class _Sched:
    pass


def _prepare(row, col, vals, n_nodes, ncores, split, gsz):
    """Sort/pad edges into an SPMD-uniform static schedule.

    Returns (sched, per_core) where per_core[c] holds idx/dst/val arrays.
    """
    shard = n_nodes // ncores
    nb = (shard + 127) // 128

    core = row // shard
    lb = (row % shard) // 128
    dst = (row % shard) % 128
    half = (col >= split).astype(np.int64)

    order = np.lexsort((col, half, lb, core))
    core_s, lb_s, dst_s, half_s = core[order], lb[order], dst[order], half[order]
    col_s, val_s = col[order], vals[order]

    # composite key for boundary lookup
    key = (core_s * nb + lb_s) * 2 + half_s
    bounds = np.searchsorted(key, np.arange(ncores * nb * 2 + 1))

    def cnt(c, b, h):
        k = (c * nb + b) * 2 + h
        return bounds[k + 1] - bounds[k]

    # chunks per (block, half): shared across cores (SPMD)
    CH = np.zeros((nb, 2), np.int64)
    for b in range(nb):
        for h in range(2):
            m = max(cnt(c, b, h) for c in range(ncores))
            CH[b, h] = (m + 127) // 128
        if CH[b, 0] + CH[b, 1] == 0:
            CH[b, 0] = 1  # keep >=1 chunk so PSUM gets initialized

    # gather groups of gsz blocks; chunk order: (group, half, block, chunk)
    groups = []
    tot_ch = 0
    for g0 in range(0, nb, gsz):
        blocks = list(range(g0, min(g0 + gsz, nb)))
        gi = _Sched()
        gi.blocks = []
        gi.C = [0, 0]
        gi.ch0 = [0, 0]
        binfo = {b: {} for b in blocks}
        for h in range(2):
            gi.ch0[h] = tot_ch
            loff = 0
            for b in blocks:
                binfo[b][h] = (loff, int(CH[b, h]), tot_ch)
                loff += int(CH[b, h])
                tot_ch += int(CH[b, h])
            gi.C[h] = loff
        for b in blocks:
            gi.blocks.append((b, binfo[b][0], binfo[b][1]))
        groups.append(gi)

    sched = _Sched()
    sched.shard, sched.nb, sched.tot_ch, sched.groups = shard, nb, tot_ch, groups
    sched.split = split

    # per-core padded idx/dst/val arrays in the same chunk order
    per_core = []
    for c in range(ncores):
        idx = np.zeros(tot_ch * 128, np.int16)
        dstv = np.zeros(tot_ch * 128, np.float32)
        valv = np.zeros(tot_ch * 128, np.float32)
        for gi in groups:
            for b, lohh, hih in gi.blocks:
                for h, (loff, chn, ch0) in ((0, lohh), (1, hih)):
                    if chn == 0:
                        continue
                    k = (c * nb + b) * 2 + h
                    s, e = bounds[k], bounds[k + 1]
                    n = e - s
                    o = ch0 * 128
                    if n > 0:
                        cc = col_s[s:e] - (split if h else 0)
                        idx[o : o + n] = cc.astype(np.int16)
                        dstv[o : o + n] = dst_s[s:e].astype(np.float32)
                        valv[o : o + n] = val_s[s:e].astype(np.float32)
        pc = _Sched()
        pc.idx_sb = np.tile(np.ascontiguousarray(idx.reshape(-1, 16).T), (8, 1))
        pc.dst_sb = np.ascontiguousarray(dstv.reshape(tot_ch, 128).T)
        pc.val_sb = np.ascontiguousarray(valv.reshape(tot_ch, 128).T)
        per_core.append(pc)
    return sched, per_core


# ---------------------------------------------------------------------------
# Device program
# ---------------------------------------------------------------------------
def _build(sched, n_nodes, ncores, enable_asserts=False):
    nb, shard, tot_ch, split = sched.nb, sched.shard, sched.tot_ch, sched.split
    npad = nb * 128
    nhi = n_nodes - split

    nc = bacc.Bacc(
        "TRN2",
        target_bir_lowering=False,
        debug=False,
        enable_asserts=enable_asserts,
        num_devices=ncores,
    )

    xT_d = nc.declare_dram_parameter("xT", [128, npad], FP16, isOutput=False)
    idx_d = nc.declare_dram_parameter("idx", [128, tot_ch * 8], I16, isOutput=False)
    dst_d = nc.declare_dram_parameter("dst", [128, tot_ch], F32, isOutput=False)
    val_d = nc.declare_dram_parameter("val", [128, tot_ch], F32, isOutput=False)
    iota_d = nc.declare_dram_parameter("iota", [128, 128], FP16, isOutput=False)
    w_d = nc.declare_dram_parameter("w", [128, 3, 128], FP16, isOutput=False)
    b_d = nc.declare_dram_parameter("b", [128, 3], F32, isOutput=False)
    fw1_d = nc.declare_dram_parameter("fw1", [128, 3, 128], FP16, isOutput=False)
    fb1_d = nc.declare_dram_parameter("fb1", [128, 1], F32, isOutput=False)
    fw2_d = nc.declare_dram_parameter("fw2", [128, 64], FP16, isOutput=False)
    fb2_d = nc.declare_dram_parameter("fb2", [64, 1], F32, isOutput=False)
    fw3_d = nc.declare_dram_parameter("fw3", [64, 2], FP16, isOutput=False)
    fb3_d = nc.declare_dram_parameter("fb3", [2, 1], F32, isOutput=False)
    out_d = nc.declare_dram_parameter("out", [2, npad], F32, isOutput=True)

    Relu = mybir.ActivationFunctionType.Relu
    Copy = mybir.ActivationFunctionType.Copy
    Ident = mybir.ActivationFunctionType.Identity
    iseq = mybir.AluOpType.is_equal
    mult = mybir.AluOpType.mult

    with tile.TileContext(nc) as tc:
        with (
            tc.tile_pool(name="const", bufs=1) as cpool,
            tc.tile_pool(name="dram", bufs=1, space="DRAM") as dpool,
            tc.tile_pool(name="work", bufs=3) as wpool,
            tc.tile_pool(name="sbuild", bufs=6) as spool,
            tc.tile_pool(name="psum", bufs=2, space="PSUM") as ppool,
        ):
            sup_ts = [
                dpool.tile([shard, 128], FP16, name=f"sup_sh{l}", tag=f"sup_sh{l}")
                for l in range(3)
            ]
            tbl_ts = [
                dpool.tile([n_nodes, 128], FP16, addr_space="Shared", name=f"tbl{l}", tag=f"tbl{l}")
                for l in range(3)
            ]

            def load(d, shape, dtype, name):
                t = cpool.tile(list(shape), dtype, name=name)
                nc.sync.dma_start(t[:], d[:])
                return t

            xT = load(xT_d, [128, npad], FP16, "xT")
            idxs = load(idx_d, [128, tot_ch * 8], I16, "idxs")
            dstv = load(dst_d, [128, tot_ch], F32, "dstv")
            valv = load(val_d, [128, tot_ch], F32, "valv")
            iota = load(iota_d, [128, 128], FP16, "iota")
            w = load(w_d, [128, 3, 128], FP16, "w")
            bl = load(b_d, [128, 3], F32, "bl")
            fw1 = load(fw1_d, [128, 3, 128], FP16, "fw1")
            fb1 = load(fb1_d, [128, 1], F32, "fb1")
            fw2 = load(fw2_d, [128, 64], FP16, "fw2")
            fb2 = load(fb2_d, [64, 1], F32, "fb2")
            fw3 = load(fw3_d, [64, 2], FP16, "fw3")
            fb3 = load(fb3_d, [2, 1], F32, "fb3")

            gT = [cpool.tile([128, npad], FP16, name=f"gT{l}") for l in range(3)]
            outT = cpool.tile([2, npad], F32, name="outT")

            prev = xT
            for l in range(3):
                sup_t = sup_ts[l]
                tbl_t = tbl_ts[l]
                # ---- local GEMM: support = g_prev @ W_l (node-major psum) --
                for ib in range(nb):
                    ps = ppool.tile([128, 128], F32, tag="sup", name="ps_sup")
                    nc.tensor.matmul(
                        ps[:],
                        prev[:, ib * 128 : (ib + 1) * 128],
                        w[:, l, :],
                        start=True,
                        stop=True,
                    )
                    sup_sb = wpool.tile([128, 128], FP16, tag="sup_sb", name="sup_sb")
                    nc.scalar.activation(sup_sb[:], ps[:], Copy)
                    rows = min(128, shard - ib * 128)
                    nc.sync.dma_start(
                        sup_t[ib * 128 : ib * 128 + rows, :], sup_sb[:rows, :]
                    )

                # ---- AllGather the support table ---------------------------
                nc.gpsimd.collective_compute(
                    "AllGather",
                    mybir.AluOpType.bypass,
                    replica_groups=[list(range(ncores))],
                    ins=[sup_t.opt()],
                    outs=[tbl_t.opt()],
                )

                # ---- gather + segment-sum per destination block ------------
                for gi in sched.groups:
                    mt = {}
                    for h in range(2):
                        C = gi.C[h]
                        if C == 0:
                            continue
                        m = wpool.tile(
                            [128, C * 128], FP16, tag=f"msgs{h}", name=f"msgs{h}", bufs=2
                        )
                        base, span = (0, min(split, n_nodes)) if h == 0 else (split, nhi)
                        m3d = m[:].rearrange("p (c e) -> p c e", e=128)
                        # cap per-call num_idxs (large single calls hang on HW)
                        MAXG = 6
                        for c0 in range(0, C, MAXG):
                            cn = min(MAXG, C - c0)
                            nc.gpsimd.dma_gather(
                                out_ap=m3d[:, c0 : c0 + cn, :],
                                in_ap=tbl_t[base : base + span, :],
                                idxs_ap=idxs[
                                    :,
                                    (gi.ch0[h] + c0) * 8 : (gi.ch0[h] + c0 + cn) * 8,
                                ],
                                num_idxs=cn * 128,
                                num_idxs_reg=cn * 128,
                                elem_size=128,
                            )
                        mt[h] = m
                    for b, lohh, hih in gi.blocks:
                        ps = ppool.tile([128, 128], F32, tag="agg", name="ps_agg")
                        total = lohh[1] + hih[1]
                        k = 0
                        for h, (loff, chn, ch0) in ((0, lohh), (1, hih)):
                            if chn == 0:
                                continue
                            m3 = mt[h][:].rearrange("p (c e) -> p c e", e=128)
                            for i in range(chn):
                                S = spool.tile([128, 128], FP16, tag="S", name="S")
                                nc.vector.tensor_scalar(
                                    S[:],
                                    iota[:],
                                    dstv[:, ch0 + i : ch0 + i + 1],
                                    valv[:, ch0 + i : ch0 + i + 1],
                                    iseq,
                                    mult,
                                )
                                nc.tensor.matmul(
                                    ps[:],
                                    m3[:, loff + i, :],
                                    S[:],
                                    start=(k == 0),
                                    stop=(k == total - 1),
                                )
                                k += 1
                        nc.scalar.activation(
                            gT[l][:, b * 128 : (b + 1) * 128],
                            ps[:],
                            Relu,
                            bias=bl[:, l : l + 1],
                        )
                prev = gT[l]

            # ---- FC head (all feature-major) -------------------------------
            for ib in range(nb):
                sl = slice(ib * 128, (ib + 1) * 128)
                ps1 = ppool.tile([128, 128], F32, tag="fc1", name="ps_fc1", bufs=1)
                for j in range(3):
                    nc.tensor.matmul(
                        ps1[:], fw1[:, j, :], gT[j][:, sl], start=(j == 0), stop=(j == 2)
                    )
                h1 = wpool.tile([128, 128], FP16, tag="h1", name="h1")
                nc.scalar.activation(h1[:], ps1[:], Relu, bias=fb1[:, 0:1])
                ps2 = ppool.tile([64, 128], F32, tag="fc2", name="ps_fc2", bufs=1)
                nc.tensor.matmul(ps2[:], fw2[:], h1[:], start=True, stop=True)
                h2 = wpool.tile([64, 128], FP16, tag="h2", name="h2")
                nc.scalar.activation(h2[:], ps2[:], Relu, bias=fb2[:])
                ps3 = ppool.tile([2, 128], F32, tag="fc3", name="ps_fc3", bufs=1)
                nc.tensor.matmul(ps3[:], fw3[:], h2[:], start=True, stop=True)
                nc.scalar.activation(outT[:, sl], ps3[:], Ident, bias=fb3[:])

            nc.sync.dma_start(out_d[:], outT[:])

    nc.compile()
    return nc


# ---------------------------------------------------------------------------
# Input packing
# ---------------------------------------------------------------------------
def _in_maps(inputs, sched, per_core, n_nodes, ncores):
    shard, npad = sched.shard, sched.nb * 128
    X = np.asarray(inputs["input_feature"], np.float32)
    xTs = []
    for c in range(ncores):
        xt = np.zeros((128, npad), np.float16)
        xt[:, :shard] = X[c * shard : (c + 1) * shard].T.astype(np.float16)
        xTs.append(xt)

    f16 = lambda a: np.ascontiguousarray(np.asarray(a, np.float32).astype(np.float16))
    f32 = lambda a: np.ascontiguousarray(np.asarray(a, np.float32))
    com = {
        "iota": np.ascontiguousarray(
            np.broadcast_to(np.arange(128, dtype=np.float16), (128, 128))
        ),
        "w": np.stack([f16(inputs[k]) for k in ("W1", "W2", "W3")], axis=1),
        "b": np.stack([f32(inputs[k]) for k in ("b1", "b2", "b3")], axis=1),
        "fw1": np.ascontiguousarray(
            f16(inputs["fcW1"]).reshape(3, 128, 128).transpose(1, 0, 2)
        ),
        "fb1": f32(inputs["fcb1"]).reshape(128, 1),
        "fw2": f16(inputs["fcW2"]),
        "fb2": f32(inputs["fcb2"]).reshape(64, 1),
        "fw3": f16(inputs["fcW3"]),
        "fb3": f32(inputs["fcb3"]).reshape(2, 1),
    }
    maps = []
    for c in range(ncores):
        m = dict(com)
        m["xT"] = xTs[c]
        m["idx"] = per_core[c].idx_sb
        m["dst"] = per_core[c].dst_sb
        m["val"] = per_core[c].val_sb
        maps.append(m)
    return maps


def _postprocess(results, sched, ncores):
    shard = sched.shard
    outs = [np.asarray(results[c]["out"], np.float32)[:, :shard].T for c in range(ncores)]
    return np.ascontiguousarray(np.concatenate(outs, axis=0))


# ---------------------------------------------------------------------------
# Public entry point
# ---------------------------------------------------------------------------
_CACHE = {}


def _run(inputs, n_nodes, ncores, split, gsz, runner=None, enable_asserts=False, trace=False):
    row = np.asarray(inputs["adj_row"]).astype(np.int64)
    col = np.asarray(inputs["adj_col"]).astype(np.int64)
    vals = np.asarray(inputs["adj_vals"], np.float32)
    sched, per_core = _prepare(row, col, vals, n_nodes, ncores, split, gsz)
    nc = _build(sched, n_nodes, ncores, enable_asserts=enable_asserts)
    maps = _in_maps(inputs, sched, per_core, n_nodes, ncores)
    _CACHE["nc"], _CACHE["maps"] = nc, maps
    if runner is None:
        res = run_bass_kernel_spmd(nc, maps, list(range(ncores)), trace=trace)
        results = res.results
        _CACHE["last_bench"] = res
    else:
        results = runner(nc, maps)
    return _postprocess(results, sched, ncores)


def kernel(**inputs):
    return _run(inputs, N_NODES, N_CORES, SPLIT, gsz=7)



# revision 3
# speedup vs baseline: 984.2935x; 984.2935x over previous
"""GCN (3-layer graph conv + 3-layer MLP head) on 8 TRN2 NeuronCores.

Strategy (graph/1D-row parallel, per sharding hint):
  - Nodes are row-sharded across the 8 cores (6250 rows each).
  - Per layer: local GEMM support = g_prev @ W (node shard), AllGather the
    [50000,128] fp16 support table to every core, then each core aggregates
    its destination rows: for each 128-row destination block, gather the
    neighbor rows (dma_gather, int16 indices, table split at row 32768 so
    indices fit int16), and accumulate
    aggT[feat,dst] += msgs[e,feat].T @ S[e,dst] on the tensor engine with
    f32 PSUM. The one-hot scatter matrices S (S[e,dst]=val[e]) are STATIC
    (adjacency doesn't change), so they are precomputed on the host and
    streamed from DRAM — no per-chunk DVE work on device.
  - dma_gather descriptor generation runs on Q7 core pair {2q, 2q+1} for
    swdge queue q; calls are round-robined over 4 queues so all 8 Q7 cores
    generate descriptors in parallel.
  - Everything stays feature-major (gT = [feat, node]) so no transposes are
    ever needed; the FC head runs the same way and the [2, n] logits are
    transposed back on the host.

Numerics: fp16 storage / f32 accumulation -> ~2.6e-3 norm rel err vs the
f32 reference.
"""

import numpy as np

import concourse.bass as bass
import concourse.bacc as bacc
import concourse.mybir as mybir
import concourse.tile as tile
from concourse.bass_utils import run_bass_kernel_spmd

FP16 = mybir.dt.float16
F32 = mybir.dt.float32
I16 = mybir.dt.int16

N_NODES = 50000
N_CORES = 8
D = 128
SPLIT = 32768  # int16 gather-index limit: table rows >= SPLIT use a 2nd base
NQUEUES = 4  # SWDGE queues; descriptor gen parallelism across Q7 core pairs
MAXG = 6  # chunks per dma_gather call (768 indices)


# ---------------------------------------------------------------------------
# Host-side schedule construction
# ---------------------------------------------------------------------------
class _Sched:
    pass


def _prepare(row, col, vals, n_nodes, ncores, split, gsz):
    """Sort/pad edges into an SPMD-uniform static schedule.

    Returns (sched, per_core) where per_core[c] holds idx/S arrays.
    """
    shard = n_nodes // ncores
    nb = (shard + 127) // 128

    core = row // shard
    lb = (row % shard) // 128
    dst = (row % shard) % 128
    half = (col >= split).astype(np.int64)

    order = np.lexsort((col, half, lb, core))
    core_s, lb_s, dst_s, half_s = core[order], lb[order], dst[order], half[order]
    col_s, val_s = col[order], vals[order]

    # composite key for boundary lookup
    key = (core_s * nb + lb_s) * 2 + half_s
    bounds = np.searchsorted(key, np.arange(ncores * nb * 2 + 1))

    def cnt(c, b, h):
        k = (c * nb + b) * 2 + h
        return bounds[k + 1] - bounds[k]

    # chunks per (block, half): shared across cores (SPMD)
    CH = np.zeros((nb, 2), np.int64)
    for b in range(nb):
        for h in range(2):
            m = max(cnt(c, b, h) for c in range(ncores))
            CH[b, h] = (m + 127) // 128
        if CH[b, 0] + CH[b, 1] == 0:
            CH[b, 0] = 1  # keep >=1 chunk so PSUM gets initialized

    # gather groups of gsz blocks; chunk order: (group, half, block, chunk)
    groups = []
    tot_ch = 0
    for g0 in range(0, nb, gsz):
        blocks = list(range(g0, min(g0 + gsz, nb)))
        gi = _Sched()
        gi.blocks = []
        gi.C = [0, 0]
        gi.ch0 = [0, 0]
        binfo = {b: {} for b in blocks}
        for h in range(2):
            gi.ch0[h] = tot_ch
            loff = 0
            for b in blocks:
                binfo[b][h] = (loff, int(CH[b, h]), tot_ch)
                loff += int(CH[b, h])
                tot_ch += int(CH[b, h])
            gi.C[h] = loff
        for b in blocks:
            gi.blocks.append((b, binfo[b][0], binfo[b][1]))
        groups.append(gi)

    sched = _Sched()
    sched.shard, sched.nb, sched.tot_ch, sched.groups = shard, nb, tot_ch, groups
    sched.split = split

    # per-core padded idx + host-built one-hot scatter tiles, same chunk order
    per_core = []
    for c in range(ncores):
        idx = np.zeros(tot_ch * 128, np.int16)
        dstv = np.zeros(tot_ch * 128, np.int64)
        valv = np.zeros(tot_ch * 128, np.float32)
        filled = np.zeros(tot_ch * 128, np.bool_)
        for gi in groups:
            for b, lohh, hih in gi.blocks:
                for h, (loff, chn, ch0) in ((0, lohh), (1, hih)):
                    if chn == 0:
                        continue
                    k = (c * nb + b) * 2 + h
                    s, e = bounds[k], bounds[k + 1]
                    n = e - s
                    o = ch0 * 128
                    if n > 0:
                        cc = col_s[s:e] - (split if h else 0)
                        idx[o : o + n] = cc.astype(np.int16)
                        dstv[o : o + n] = dst_s[s:e]
                        valv[o : o + n] = val_s[s:e]
                        filled[o : o + n] = True
        pc = _Sched()
        pc.idx_sb = np.tile(np.ascontiguousarray(idx.reshape(-1, 16).T), (8, 1))
        # S[p, ch, dst] = val for edge (ch, p); zero rows for padding
        S = np.zeros((tot_ch * 128, 128), np.float16)
        S[np.arange(tot_ch * 128)[filled], dstv[filled]] = valv[filled].astype(
            np.float16
        )
        # -> [128 partitions(e), tot_ch, 128(dst)]
        pc.s_host = np.ascontiguousarray(
            S.reshape(tot_ch, 128, 128).transpose(1, 0, 2)
        )
        per_core.append(pc)
    return sched, per_core


# ---------------------------------------------------------------------------
# Device program
# ---------------------------------------------------------------------------
def _build(sched, n_nodes, ncores, enable_asserts=False):
    nb, shard, tot_ch, split = sched.nb, sched.shard, sched.tot_ch, sched.split
    npad = nb * 128
    nhi = n_nodes - split

    nc = bacc.Bacc(
        "TRN2",
        target_bir_lowering=False,
        debug=False,
        enable_asserts=enable_asserts,
        num_devices=ncores,
        num_swdge_queues=NQUEUES,
    )

    xT_d = nc.declare_dram_parameter("xT", [128, npad], FP16, isOutput=False)
    idx_d = nc.declare_dram_parameter("idx", [128, tot_ch * 8], I16, isOutput=False)
    s_d = nc.declare_dram_parameter("s", [128, tot_ch, 128], FP16, isOutput=False)
    w_d = nc.declare_dram_parameter("w", [128, 3, 128], FP16, isOutput=False)
    b_d = nc.declare_dram_parameter("b", [128, 3], F32, isOutput=False)
    fw1_d = nc.declare_dram_parameter("fw1", [128, 3, 128], FP16, isOutput=False)
    fb1_d = nc.declare_dram_parameter("fb1", [128, 1], F32, isOutput=False)
    fw2_d = nc.declare_dram_parameter("fw2", [128, 64], FP16, isOutput=False)
    fb2_d = nc.declare_dram_parameter("fb2", [64, 1], F32, isOutput=False)
    fw3_d = nc.declare_dram_parameter("fw3", [64, 2], FP16, isOutput=False)
    fb3_d = nc.declare_dram_parameter("fb3", [2, 1], F32, isOutput=False)
    out_d = nc.declare_dram_parameter("out", [2, npad], F32, isOutput=True)

    Relu = mybir.ActivationFunctionType.Relu
    Copy = mybir.ActivationFunctionType.Copy
    Ident = mybir.ActivationFunctionType.Identity

    with tile.TileContext(nc) as tc:
        with (
            tc.tile_pool(name="const", bufs=1) as cpool,
            tc.tile_pool(name="dram", bufs=1, space="DRAM") as dpool,
            tc.tile_pool(name="work", bufs=3) as wpool,
            tc.tile_pool(name="psum", bufs=2, space="PSUM") as ppool,
        ):
            sup_ts = [
                dpool.tile([shard, 128], FP16, name=f"sup_sh{l}", tag=f"sup_sh{l}")
                for l in range(3)
            ]
            tbl_ts = [
                dpool.tile([n_nodes, 128], FP16, addr_space="Shared", name=f"tbl{l}", tag=f"tbl{l}")
                for l in range(3)
            ]

            def load(d, shape, dtype, name):
                t = cpool.tile(list(shape), dtype, name=name)
                nc.sync.dma_start(t[:], d[:])
                return t

            xT = load(xT_d, [128, npad], FP16, "xT")
            idxs = load(idx_d, [128, tot_ch * 8], I16, "idxs")
            w = load(w_d, [128, 3, 128], FP16, "w")
            bl = load(b_d, [128, 3], F32, "bl")
            fw1 = load(fw1_d, [128, 3, 128], FP16, "fw1")
            fb1 = load(fb1_d, [128, 1], F32, "fb1")
            fw2 = load(fw2_d, [128, 64], FP16, "fw2")
            fb2 = load(fb2_d, [64, 1], F32, "fb2")
            fw3 = load(fw3_d, [64, 2], FP16, "fw3")
            fb3 = load(fb3_d, [2, 1], F32, "fb3")

            gT = [cpool.tile([128, npad], FP16, name=f"gT{l}") for l in range(3)]
            outT = cpool.tile([2, npad], F32, name="outT")

            qctr = 0  # round-robin dma_gather calls over SWDGE queues
            prev = xT
            for l in range(3):
                sup_t = sup_ts[l]
                tbl_t = tbl_ts[l]
                # ---- local GEMM: support = g_prev @ W_l (node-major psum) --
                for ib in range(nb):
                    ps = ppool.tile([128, 128], F32, tag="sup", name="ps_sup")
                    nc.tensor.matmul(
                        ps[:],
                        prev[:, ib * 128 : (ib + 1) * 128],
                        w[:, l, :],
                        start=True,
                        stop=True,
                    )
                    sup_sb = wpool.tile([128, 128], FP16, tag="sup_sb", name="sup_sb")
                    nc.scalar.activation(sup_sb[:], ps[:], Copy)
                    rows = min(128, shard - ib * 128)
                    nc.sync.dma_start(
                        sup_t[ib * 128 : ib * 128 + rows, :], sup_sb[:rows, :]
                    )

                # ---- AllGather the support table ---------------------------
                nc.gpsimd.collective_compute(
                    "AllGather",
                    mybir.AluOpType.bypass,
                    replica_groups=[list(range(ncores))],
                    ins=[sup_t.opt()],
                    outs=[tbl_t.opt()],
                )

                # ---- gather + segment-sum per destination block ------------
                for gi in sched.groups:
                    gtot = gi.C[0] + gi.C[1]
                    # one DMA brings the whole group's precomputed scatter
                    # matrices (chunk-major, contiguous)
                    stile = wpool.tile(
                        [128, gtot, 128], FP16, tag="S", name="S", bufs=2
                    )
                    nc.sync.dma_start(
                        stile[:], s_d[:, gi.ch0[0] : gi.ch0[0] + gtot, :]
                    )
                    mt = {}
                    for h in range(2):
                        C = gi.C[h]
                        if C == 0:
                            continue
                        m = wpool.tile(
                            [128, C * 128], FP16, tag=f"msgs{h}", name=f"msgs{h}", bufs=2
                        )
                        base, span = (0, min(split, n_nodes)) if h == 0 else (split, nhi)
                        m3d = m[:].rearrange("p (c e) -> p c e", e=128)
                        # cap per-call num_idxs (large single calls hang on HW)
                        for c0 in range(0, C, MAXG):
                            cn = min(MAXG, C - c0)
                            nc.gpsimd.dma_gather(
                                out_ap=m3d[:, c0 : c0 + cn, :],
                                in_ap=tbl_t[base : base + span, :],
                                idxs_ap=idxs[
                                    :,
                                    (gi.ch0[h] + c0) * 8 : (gi.ch0[h] + c0 + cn) * 8,
                                ],
                                num_idxs=cn * 128,
                                num_idxs_reg=cn * 128,
                                elem_size=128,
                                queue_num=qctr % NQUEUES,
                            )
                            qctr += 1
                        mt[h] = m
                    for b, lohh, hih in gi.blocks:
                        ps = ppool.tile([128, 128], F32, tag="agg", name="ps_agg")
                        total = lohh[1] + hih[1]
                        k = 0
                        for h, (loff, chn, ch0) in ((0, lohh), (1, hih)):
                            if chn == 0:
                                continue
                            m3 = mt[h][:].rearrange("p (c e) -> p c e", e=128)
                            for i in range(chn):
                                nc.tensor.matmul(
                                    ps[:],
                                    m3[:, loff + i, :],
                                    stile[:, ch0 + i - gi.ch0[0], :],
                                    start=(k == 0),
                                    stop=(k == total - 1),
                                )
                                k += 1
                        nc.scalar.activation(
                            gT[l][:, b * 128 : (b + 1) * 128],
                            ps[:],
                            Relu,
                            bias=bl[:, l : l + 1],
                        )
                prev = gT[l]

            # ---- FC head (all feature-major) -------------------------------
            for ib in range(nb):
                sl = slice(ib * 128, (ib + 1) * 128)
                ps1 = ppool.tile([128, 128], F32, tag="fc1", name="ps_fc1", bufs=1)
                for j in range(3):
                    nc.tensor.matmul(
                        ps1[:], fw1[:, j, :], gT[j][:, sl], start=(j == 0), stop=(j == 2)
                    )
                h1 = wpool.tile([128, 128], FP16, tag="h1", name="h1")
                nc.scalar.activation(h1[:], ps1[:], Relu, bias=fb1[:, 0:1])
                ps2 = ppool.tile([64, 128], F32, tag="fc2", name="ps_fc2", bufs=1)
                nc.tensor.matmul(ps2[:], fw2[:], h1[:], start=True, stop=True)
                h2 = wpool.tile([64, 128], FP16, tag="h2", name="h2")
                nc.scalar.activation(h2[:], ps2[:], Relu, bias=fb2[:])
                ps3 = ppool.tile([2, 128], F32, tag="fc3", name="ps_fc3", bufs=1)
                nc.tensor.matmul(ps3[:], fw3[:], h2[:], start=True, stop=True)
                nc.scalar.activation(outT[:, sl], ps3[:], Ident, bias=fb3[:])

            nc.sync.dma_start(out_d[:], outT[:])

    nc.compile()
    return nc


# ---------------------------------------------------------------------------
# Input packing
# ---------------------------------------------------------------------------
def _in_maps(inputs, sched, per_core, n_nodes, ncores):
    shard, npad = sched.shard, sched.nb * 128
    X = np.asarray(inputs["input_feature"], np.float32)
    xTs = []
    for c in range(ncores):
        xt = np.zeros((128, npad), np.float16)
        xt[:, :shard] = X[c * shard : (c + 1) * shard].T.astype(np.float16)
        xTs.append(xt)

    f16 = lambda a: np.ascontiguousarray(np.asarray(a, np.float32).astype(np.float16))
    f32 = lambda a: np.ascontiguousarray(np.asarray(a, np.float32))
    com = {
        "w": np.stack([f16(inputs[k]) for k in ("W1", "W2", "W3")], axis=1),
        "b": np.stack([f32(inputs[k]) for k in ("b1", "b2", "b3")], axis=1),
        "fw1": np.ascontiguousarray(
            f16(inputs["fcW1"]).reshape(3, 128, 128).transpose(1, 0, 2)
        ),
        "fb1": f32(inputs["fcb1"]).reshape(128, 1),
        "fw2": f16(inputs["fcW2"]),
        "fb2": f32(inputs["fcb2"]).reshape(64, 1),
        "fw3": f16(inputs["fcW3"]),
        "fb3": f32(inputs["fcb3"]).reshape(2, 1),
    }
    maps = []
    for c in range(ncores):
        m = dict(com)
        m["xT"] = xTs[c]
        m["idx"] = per_core[c].idx_sb
        m["s"] = per_core[c].s_host
        maps.append(m)
    return maps


def _postprocess(results, sched, ncores):
    shard = sched.shard
    outs = [np.asarray(results[c]["out"], np.float32)[:, :shard].T for c in range(ncores)]
    return np.ascontiguousarray(np.concatenate(outs, axis=0))


# ---------------------------------------------------------------------------
# Public entry point
# ---------------------------------------------------------------------------
_CACHE = {}


def _run(inputs, n_nodes, ncores, split, gsz, runner=None, enable_asserts=False, trace=False):
    row = np.asarray(inputs["adj_row"]).astype(np.int64)
    col = np.asarray(inputs["adj_col"]).astype(np.int64)
    vals = np.asarray(inputs["adj_vals"], np.float32)
    sched, per_core = _prepare(row, col, vals, n_nodes, ncores, split, gsz)
    nc = _build(sched, n_nodes, ncores, enable_asserts=enable_asserts)
    maps = _in_maps(inputs, sched, per_core, n_nodes, ncores)
    _CACHE["nc"], _CACHE["maps"] = nc, maps
    if runner is None:
        res = run_bass_kernel_spmd(nc, maps, list(range(ncores)), trace=trace)
        results = res.results
        _CACHE["last_bench"] = res
    else:
        results = runner(nc, maps)
    return _postprocess(results, sched, ncores)


def kernel(**inputs):
    return _run(inputs, N_NODES, N_CORES, SPLIT, gsz=4)


# revision 7
# speedup vs baseline: 1018.4688x; 1.0347x over previous
"""GCN (3-layer graph conv + 3-layer MLP head) on 8 TRN2 NeuronCores.

Strategy (graph/1D-row parallel, per sharding hint):
  - Nodes are row-sharded across the 8 cores (6250 rows each).
  - Per layer: local GEMM support = g_prev @ W (node shard), AllGather the
    [50000,128] fp16 support table to every core, then each core aggregates
    its destination rows: for each 128-row destination block, gather the
    neighbor rows (dma_gather, int16 indices, table split at row 32768 so
    indices fit int16), and accumulate
    aggT[feat,dst] += msgs[e,feat].T @ S[e,dst] on the tensor engine with
    f32 PSUM. The one-hot scatter matrices S (S[e,dst]=val[e]) are STATIC
    (adjacency doesn't change), so they are precomputed on the host and
    streamed from DRAM — no per-chunk DVE work on device.
  - dma_gather descriptor generation runs on Q7 core pair {2q, 2q+1} for
    swdge queue q; calls are round-robined over 4 queues so all 8 Q7 cores
    generate descriptors in parallel.
  - Everything stays feature-major (gT = [feat, node]) so no transposes are
    ever needed; the FC head runs the same way and the [2, n] logits are
    transposed back on the host.

Numerics: fp16 storage / f32 accumulation -> ~2.6e-3 norm rel err vs the
f32 reference.
"""

import numpy as np

import concourse.bass as bass
import concourse.bacc as bacc
import concourse.mybir as mybir
import concourse.tile as tile
from concourse.bass_utils import run_bass_kernel_spmd

FP16 = mybir.dt.float16
F32 = mybir.dt.float32
I16 = mybir.dt.int16

N_NODES = 50000
N_CORES = 8
D = 128
SPLIT = 32768  # int16 gather-index limit: table rows >= SPLIT use a 2nd base
NQUEUES = 4  # SWDGE queues; descriptor gen parallelism across Q7 core pairs
MAXG = 6  # chunks per dma_gather call (768 indices; 49 descs fits the 64-desc ring)


# ---------------------------------------------------------------------------
# Host-side schedule construction
# ---------------------------------------------------------------------------
class _Sched:
    pass


def _prepare(row, col, vals, n_nodes, ncores, split, gsz):
    """Sort/pad edges into an SPMD-uniform static schedule.

    Returns (sched, per_core) where per_core[c] holds idx/S arrays.
    """
    shard = n_nodes // ncores
    nb = (shard + 127) // 128

    core = row // shard
    lb = (row % shard) // 128
    dst = (row % shard) % 128
    half = (col >= split).astype(np.int64)

    order = np.lexsort((col, half, lb, core))
    core_s, lb_s, dst_s, half_s = core[order], lb[order], dst[order], half[order]
    col_s, val_s = col[order], vals[order]

    # composite key for boundary lookup
    key = (core_s * nb + lb_s) * 2 + half_s
    bounds = np.searchsorted(key, np.arange(ncores * nb * 2 + 1))

    def cnt(c, b, h):
        k = (c * nb + b) * 2 + h
        return bounds[k + 1] - bounds[k]

    # chunks per (block, half): shared across cores (SPMD)
    CH = np.zeros((nb, 2), np.int64)
    for b in range(nb):
        for h in range(2):
            m = max(cnt(c, b, h) for c in range(ncores))
            CH[b, h] = (m + 127) // 128
        if CH[b, 0] + CH[b, 1] == 0:
            CH[b, 0] = 1  # keep >=1 chunk so PSUM gets initialized

    # gather groups of gsz blocks; chunk order: (group, half, block, chunk)
    groups = []
    tot_ch = 0
    for g0 in range(0, nb, gsz):
        blocks = list(range(g0, min(g0 + gsz, nb)))
        gi = _Sched()
        gi.blocks = []
        gi.C = [0, 0]
        gi.ch0 = [0, 0]
        binfo = {b: {} for b in blocks}
        for h in range(2):
            gi.ch0[h] = tot_ch
            loff = 0
            for b in blocks:
                binfo[b][h] = (loff, int(CH[b, h]), tot_ch)
                loff += int(CH[b, h])
                tot_ch += int(CH[b, h])
            gi.C[h] = loff
        for b in blocks:
            gi.blocks.append((b, binfo[b][0], binfo[b][1]))
        groups.append(gi)

    sched = _Sched()
    sched.shard, sched.nb, sched.tot_ch, sched.groups = shard, nb, tot_ch, groups
    sched.split = split

    # per-core padded idx + host-built one-hot scatter tiles, same chunk order
    per_core = []
    for c in range(ncores):
        idx = np.zeros(tot_ch * 128, np.int16)
        dstv = np.zeros(tot_ch * 128, np.int64)
        valv = np.zeros(tot_ch * 128, np.float32)
        filled = np.zeros(tot_ch * 128, np.bool_)
        for gi in groups:
            for b, lohh, hih in gi.blocks:
                for h, (loff, chn, ch0) in ((0, lohh), (1, hih)):
                    if chn == 0:
                        continue
                    k = (c * nb + b) * 2 + h
                    s, e = bounds[k], bounds[k + 1]
                    n = e - s
                    o = ch0 * 128
                    if n > 0:
                        cc = col_s[s:e] - (split if h else 0)
                        idx[o : o + n] = cc.astype(np.int16)
                        dstv[o : o + n] = dst_s[s:e]
                        valv[o : o + n] = val_s[s:e]
                        filled[o : o + n] = True
        pc = _Sched()
        pc.idx_sb = np.tile(np.ascontiguousarray(idx.reshape(-1, 16).T), (8, 1))
        # S[p, ch, dst] = val for edge (ch, p); zero rows for padding
        S = np.zeros((tot_ch * 128, 128), np.float16)
        S[np.arange(tot_ch * 128)[filled], dstv[filled]] = valv[filled].astype(
            np.float16
        )
        # -> [128 partitions(e), tot_ch, 128(dst)]
        pc.s_host = np.ascontiguousarray(
            S.reshape(tot_ch, 128, 128).transpose(1, 0, 2)
        )
        per_core.append(pc)
    return sched, per_core


# ---------------------------------------------------------------------------
# Device program
# ---------------------------------------------------------------------------
def _build(sched, n_nodes, ncores, enable_asserts=False):
    nb, shard, tot_ch, split = sched.nb, sched.shard, sched.tot_ch, sched.split
    npad = nb * 128
    nhi = n_nodes - split

    nc = bacc.Bacc(
        "TRN2",
        target_bir_lowering=False,
        debug=False,
        enable_asserts=enable_asserts,
        num_devices=ncores,
        num_swdge_queues=NQUEUES,
    )

    xT_d = nc.declare_dram_parameter("xT", [128, npad], FP16, isOutput=False)
    idx_d = nc.declare_dram_parameter("idx", [128, tot_ch * 8], I16, isOutput=False)
    s_d = nc.declare_dram_parameter("s", [128, tot_ch, 128], FP16, isOutput=False)
    w_d = nc.declare_dram_parameter("w", [128, 3, 128], FP16, isOutput=False)
    b_d = nc.declare_dram_parameter("b", [128, 3], F32, isOutput=False)
    fw1_d = nc.declare_dram_parameter("fw1", [128, 3, 128], FP16, isOutput=False)
    fb1_d = nc.declare_dram_parameter("fb1", [128, 1], F32, isOutput=False)
    fw2_d = nc.declare_dram_parameter("fw2", [128, 64], FP16, isOutput=False)
    fb2_d = nc.declare_dram_parameter("fb2", [64, 1], F32, isOutput=False)
    fw3_d = nc.declare_dram_parameter("fw3", [64, 2], FP16, isOutput=False)
    fb3_d = nc.declare_dram_parameter("fb3", [2, 1], F32, isOutput=False)
    out_d = nc.declare_dram_parameter("out", [2, npad], F32, isOutput=True)

    Relu = mybir.ActivationFunctionType.Relu
    Copy = mybir.ActivationFunctionType.Copy
    Ident = mybir.ActivationFunctionType.Identity

    with tile.TileContext(nc) as tc:
        with (
            tc.tile_pool(name="const", bufs=1) as cpool,
            tc.tile_pool(name="dram", bufs=1, space="DRAM") as dpool,
            tc.tile_pool(name="work", bufs=3) as wpool,
            tc.tile_pool(name="psum", bufs=2, space="PSUM") as ppool,
        ):
            sup_ts = [
                dpool.tile([shard, 128], FP16, name=f"sup_sh{l}", tag=f"sup_sh{l}")
                for l in range(3)
            ]
            tbl_ts = [
                dpool.tile([n_nodes, 128], FP16, addr_space="Shared", name=f"tbl{l}", tag=f"tbl{l}")
                for l in range(3)
            ]

            def load(d, shape, dtype, name):
                t = cpool.tile(list(shape), dtype, name=name)
                nc.sync.dma_start(t[:], d[:])
                return t

            xT = load(xT_d, [128, npad], FP16, "xT")
            idxs = load(idx_d, [128, tot_ch * 8], I16, "idxs")
            w = load(w_d, [128, 3, 128], FP16, "w")
            bl = load(b_d, [128, 3], F32, "bl")
            fw1 = load(fw1_d, [128, 3, 128], FP16, "fw1")
            fb1 = load(fb1_d, [128, 1], F32, "fb1")
            fw2 = load(fw2_d, [128, 64], FP16, "fw2")
            fb2 = load(fb2_d, [64, 1], F32, "fb2")
            fw3 = load(fw3_d, [64, 2], FP16, "fw3")
            fb3 = load(fb3_d, [2, 1], F32, "fb3")

            gT = [cpool.tile([128, npad], FP16, name=f"gT{l}") for l in range(3)]
            outT = cpool.tile([2, npad], F32, name="outT")

            qctr = 0  # round-robin dma_gather calls over SWDGE queues
            prev = xT
            for l in range(3):
                sup_t = sup_ts[l]
                tbl_t = tbl_ts[l]
                # ---- local GEMM: support = g_prev @ W_l (node-major psum) --
                for ib in range(nb):
                    ps = ppool.tile([128, 128], F32, tag="sup", name="ps_sup")
                    nc.tensor.matmul(
                        ps[:],
                        prev[:, ib * 128 : (ib + 1) * 128],
                        w[:, l, :],
                        start=True,
                        stop=True,
                    )
                    sup_sb = wpool.tile([128, 128], FP16, tag="sup_sb", name="sup_sb")
                    nc.scalar.activation(sup_sb[:], ps[:], Copy)
                    rows = min(128, shard - ib * 128)
                    nc.sync.dma_start(
                        sup_t[ib * 128 : ib * 128 + rows, :], sup_sb[:rows, :]
                    )

                # ---- AllGather the support table ---------------------------
                nc.gpsimd.collective_compute(
                    "AllGather",
                    mybir.AluOpType.bypass,
                    replica_groups=[list(range(ncores))],
                    ins=[sup_t.opt()],
                    outs=[tbl_t.opt()],
                )

                # ---- gather + segment-sum per destination block ------------
                for gi in sched.groups:
                    gtot = gi.C[0] + gi.C[1]
                    # one DMA brings the whole group's precomputed scatter
                    # matrices (chunk-major, contiguous)
                    stile = wpool.tile(
                        [128, gtot, 128], FP16, tag="S", name="S", bufs=3
                    )
                    nc.sync.dma_start(
                        stile[:], s_d[:, gi.ch0[0] : gi.ch0[0] + gtot, :]
                    )
                    mt = {}
                    for h in range(2):
                        C = gi.C[h]
                        if C == 0:
                            continue
                        m = wpool.tile(
                            [128, C * 128], FP16, tag=f"msgs{h}", name=f"msgs{h}", bufs=3
                        )
                        base, span = (0, min(split, n_nodes)) if h == 0 else (split, nhi)
                        m3d = m[:].rearrange("p (c e) -> p c e", e=128)
                        # cap per-call num_idxs (large single calls hang on HW)
                        for c0 in range(0, C, MAXG):
                            cn = min(MAXG, C - c0)
                            nc.gpsimd.dma_gather(
                                out_ap=m3d[:, c0 : c0 + cn, :],
                                in_ap=tbl_t[base : base + span, :],
                                idxs_ap=idxs[
                                    :,
                                    (gi.ch0[h] + c0) * 8 : (gi.ch0[h] + c0 + cn) * 8,
                                ],
                                num_idxs=cn * 128,
                                num_idxs_reg=cn * 128,
                                elem_size=128,
                                queue_num=qctr % NQUEUES,
                            )
                            qctr += 1
                        mt[h] = m
                    for b, lohh, hih in gi.blocks:
                        ps = ppool.tile([128, 128], F32, tag="agg", name="ps_agg", bufs=3)
                        total = lohh[1] + hih[1]
                        k = 0
                        for h, (loff, chn, ch0) in ((0, lohh), (1, hih)):
                            if chn == 0:
                                continue
                            m3 = mt[h][:].rearrange("p (c e) -> p c e", e=128)
                            for i in range(chn):
                                nc.tensor.matmul(
                                    ps[:],
                                    m3[:, loff + i, :],
                                    stile[:, ch0 + i - gi.ch0[0], :],
                                    start=(k == 0),
                                    stop=(k == total - 1),
                                )
                                k += 1
                        nc.scalar.activation(
                            gT[l][:, b * 128 : (b + 1) * 128],
                            ps[:],
                            Relu,
                            bias=bl[:, l : l + 1],
                        )
                prev = gT[l]

            # ---- FC head (all feature-major) -------------------------------
            for ib in range(nb):
                sl = slice(ib * 128, (ib + 1) * 128)
                ps1 = ppool.tile([128, 128], F32, tag="fc1", name="ps_fc1", bufs=1)
                for j in range(3):
                    nc.tensor.matmul(
                        ps1[:], fw1[:, j, :], gT[j][:, sl], start=(j == 0), stop=(j == 2)
                    )
                h1 = wpool.tile([128, 128], FP16, tag="h1", name="h1")
                nc.scalar.activation(h1[:], ps1[:], Relu, bias=fb1[:, 0:1])
                ps2 = ppool.tile([64, 128], F32, tag="fc2", name="ps_fc2", bufs=1)
                nc.tensor.matmul(ps2[:], fw2[:], h1[:], start=True, stop=True)
                h2 = wpool.tile([64, 128], FP16, tag="h2", name="h2")
                nc.scalar.activation(h2[:], ps2[:], Relu, bias=fb2[:])
                ps3 = ppool.tile([2, 128], F32, tag="fc3", name="ps_fc3", bufs=1)
                nc.tensor.matmul(ps3[:], fw3[:], h2[:], start=True, stop=True)
                nc.scalar.activation(outT[:, sl], ps3[:], Ident, bias=fb3[:])

            nc.sync.dma_start(out_d[:], outT[:])

    nc.compile()
    return nc


# ---------------------------------------------------------------------------
# Input packing
# ---------------------------------------------------------------------------
def _in_maps(inputs, sched, per_core, n_nodes, ncores):
    shard, npad = sched.shard, sched.nb * 128
    X = np.asarray(inputs["input_feature"], np.float32)
    xTs = []
    for c in range(ncores):
        xt = np.zeros((128, npad), np.float16)
        xt[:, :shard] = X[c * shard : (c + 1) * shard].T.astype(np.float16)
        xTs.append(xt)

    f16 = lambda a: np.ascontiguousarray(np.asarray(a, np.float32).astype(np.float16))
    f32 = lambda a: np.ascontiguousarray(np.asarray(a, np.float32))
    com = {
        "w": np.stack([f16(inputs[k]) for k in ("W1", "W2", "W3")], axis=1),
        "b": np.stack([f32(inputs[k]) for k in ("b1", "b2", "b3")], axis=1),
        "fw1": np.ascontiguousarray(
            f16(inputs["fcW1"]).reshape(3, 128, 128).transpose(1, 0, 2)
        ),
        "fb1": f32(inputs["fcb1"]).reshape(128, 1),
        "fw2": f16(inputs["fcW2"]),
        "fb2": f32(inputs["fcb2"]).reshape(64, 1),
        "fw3": f16(inputs["fcW3"]),
        "fb3": f32(inputs["fcb3"]).reshape(2, 1),
    }
    maps = []
    for c in range(ncores):
        m = dict(com)
        m["xT"] = xTs[c]
        m["idx"] = per_core[c].idx_sb
        m["s"] = per_core[c].s_host
        maps.append(m)
    return maps


def _postprocess(results, sched, ncores):
    shard = sched.shard
    outs = [np.asarray(results[c]["out"], np.float32)[:, :shard].T for c in range(ncores)]
    return np.ascontiguousarray(np.concatenate(outs, axis=0))


# ---------------------------------------------------------------------------
# Public entry point
# ---------------------------------------------------------------------------
_CACHE = {}


def _run(inputs, n_nodes, ncores, split, gsz, runner=None, enable_asserts=False, trace=False):
    row = np.asarray(inputs["adj_row"]).astype(np.int64)
    col = np.asarray(inputs["adj_col"]).astype(np.int64)
    vals = np.asarray(inputs["adj_vals"], np.float32)
    sched, per_core = _prepare(row, col, vals, n_nodes, ncores, split, gsz)
    nc = _build(sched, n_nodes, ncores, enable_asserts=enable_asserts)
    maps = _in_maps(inputs, sched, per_core, n_nodes, ncores)
    _CACHE["nc"], _CACHE["maps"] = nc, maps
    if runner is None:
        res = run_bass_kernel_spmd(nc, maps, list(range(ncores)), trace=trace)
        results = res.results
        _CACHE["last_bench"] = res
    else:
        results = runner(nc, maps)
    return _postprocess(results, sched, ncores)


def kernel(**inputs):
    return _run(inputs, N_NODES, N_CORES, SPLIT, gsz=4)
